# revision 37
# baseline (speedup 1.0000x reference)
"""Trainium2 Bass kernel for nn_BDH_90984587198975 (6-layer BDH with Hebbian
fast weights), SPMD over 8 NeuronCores.

Sharding: tensor-parallel over the flattened latent dim NHL=4*8192.  Core c
owns a 4096-wide slice of head h=c//2 (half=c%2), with lanes permuted so rope
pairs split into [even-members(2048) | odd-members(2048)] (rotation becomes a
tile swap instead of a cross-partition shuffle).  F (fast weights) stays
sharded by latent rows — its update is local.  Per layer there are exactly two
collectives: a pair AllReduce of the attention partial (L split in half within
a head) and an 8-core AllReduce of the y_mlp partial (decoder/F/Hebbian terms
contract over the latent shard).

All matmuls run in bf16 (f32 accumulation in PSUM); LayerNorm statistics and
the residual stream stay f32.
"""
import math
import numpy as np
import ml_dtypes

import concourse.bass as bass
import concourse.mybir as mybir
import concourse.tile as tile
from concourse import bacc
from concourse.masks import make_identity
from concourse.bass_utils import run_bass_kernel_spmd

BF = ml_dtypes.bfloat16
F8 = ml_dtypes.float8_e4m3
f32 = mybir.dt.float32
bf16 = mybir.dt.bfloat16
f8 = mybir.dt.float8e4
AF = mybir.ActivationFunctionType
OP = mybir.AluOpType
DR = mybir.MatmulPerfMode.DoubleRow
WSC = 32.0         # fp8 weight prescale (encoder/encoder_v/lm_head std=0.02)

N_LAYER = 6
D = 256
NH = 4
VOCAB = 130
LR = 0.01
L = 8192
EPS = 1e-5
TWO_PI = 2.0 * math.pi
THETA = 65536.0
B, T = 2, 512
NCORE = 8
SH = 4096          # latent shard per core
NLT = SH // 128    # 32 latent tiles
NBT = (B * T) // 128  # 8 bt tiles
CHK = 128          # hebbian time chunk
NCH = T // CHK     # 4 chunks

_CACHE = {}


# ----------------------------------------------------------------- builder --
def _emit(nc, n_layer, taps, ablate=None):
    # ---- DRAM I/O ----
    oh = nc.dram_tensor("onehotT", [256, 1024], f32, kind="ExternalInput")
    emb = nc.dram_tensor("embedp", [256, 256], f32, kind="ExternalInput")
    wenc = nc.dram_tensor("wenc", [NLT * 128, 256], bf16, kind="ExternalInput")
    wencv = nc.dram_tensor("wencv", [NLT * 128, 256], bf16, kind="ExternalInput")
    wdec = nc.dram_tensor("wdec", [SH, 256], bf16, kind="ExternalInput")
    trig = nc.dram_tensor("trig", [2048, 1024], bf16, kind="ExternalInput")
    msk = nc.dram_tensor("masks", [128, 384], f32, kind="ExternalInput")
    lmh = nc.dram_tensor("lmh", [256, 130], bf16, kind="ExternalInput")
    out = nc.dram_tensor("out", [1024, 130], f32, kind="ExternalOutput")
    tap_t = {}
    if taps:
        for name, shape, dt_ in [("t_x0", [128, 2048], f32), ("t_xs", [128, 1024], bf16),
                                 ("t_attn", [128, 2048], bf16), ("t_ymlp", [128, 2048], bf16),
                                 ("t_x1", [128, 2048], f32), ("t_f", [128, 8192], bf16)]:
            tap_t[name] = nc.dram_tensor(name, shape, dt_, kind="ExternalOutput")

    from contextlib import ExitStack
    tc = tile.TileContext(nc)
    with tc, ExitStack() as stk:
        per = stk.enter_context(tc.tile_pool(name="per", bufs=1))
        sm = stk.enter_context(tc.tile_pool(name="sm", bufs=2))
        qp = stk.enter_context(tc.tile_pool(name="qp", bufs=4))
        ysp = stk.enter_context(tc.tile_pool(name="ysp", bufs=3))
        wp = stk.enter_context(tc.tile_pool(name="wp", bufs=4))
        tp = stk.enter_context(tc.tile_pool(name="tp", bufs=3))
        ps = stk.enter_context(tc.tile_pool(name="ps", bufs=4, space="PSUM"))
        pacc = stk.enter_context(tc.tile_pool(name="pacc", bufs=4, space="PSUM"))
        dram = stk.enter_context(tc.tile_pool(name="dram", bufs=2, space="DRAM"))

        # ---- persistent tiles ----
        xs = per.tile([128, NLT * 1024], bf16, tag="xs")
        F = per.tile([128, NLT * 256], bf16, tag="F")
        xf = per.tile([128, 2048], f32, tag="xf")
        xbf = per.tile([128, 2048], bf16, tag="xbf")
        xTbf = per.tile([128, 2048], bf16, tag="xTbf")
        ymlp = per.tile([128, 2048], bf16, tag="ymlp")
        attn = per.tile([128, 2048], bf16, tag="attn")
        ykv = per.tile([128, 2048], bf16, tag="ykv")
        ykvT = per.tile([128, 2048], bf16, tag="ykvT")
        zst = per.tile([128, 2048], f32, tag="zst")
        ident = per.tile([128, 128], bf16, tag="ident")
        maskU = per.tile([128, 128], f32, tag="maskU")
        maskS = per.tile([128, 128], f32, tag="maskS")
        maskS0 = per.tile([128, 128], f32, tag="maskS0")
        epst = per.tile([128, 1], f32, tag="epst")
        xsh = [per.tile([128, 256], bf16, tag=f"xsh{c}", name=f"xsh{c}") for c in range(B)]
        xbt = [per.tile([128, SH], bf16, tag=f"xbt{c}", name=f"xbt{c}") for c in range(B)]
        scsb = [per.tile([128, 1280], bf16, tag=f"scsb{b}", name=f"scsb{b}") for b in range(B)]
        ssb = [per.tile([128, 256], bf16, tag=f"ssb{b}", name=f"ssb{b}") for b in range(B)]

        make_identity(nc, ident[:])
        nc.vector.memset(epst[:], EPS)
        nc.sync.dma_start(maskU[:], msk[:, 0:128])
        nc.sync.dma_start(maskS[:], msk[:, 128:256])
        nc.sync.dma_start(maskS0[:], msk[:, 256:384])
        nc.vector.memset(F[:], 0.0)

        # ---------------- helpers ----------------
        def ln_batch(t, nblk, emit_out):
            """Batched LN over `nblk` 256-wide blocks of tile t [128, nblk*256].
            emit_out(j, rstd_ap, nmr_ap) writes the normalized outputs."""
            ssum = sm.tile([128, 8], f32, tag="ssum")
            sqsum = sm.tile([128, 8], f32, tag="sqsum")
            nm2 = sm.tile([128, 8], f32, tag="nm2")
            var = sm.tile([128, 8], f32, tag="var")
            std = sm.tile([128, 8], f32, tag="std")
            rstd = sm.tile([128, 8], f32, tag="rstd")
            nmr = sm.tile([128, 8], f32, tag="nmr")
            sq = sm.tile([128, 256], f32, tag="sq")
            for j in range(nblk):
                nc.scalar.activation(sq[:], t[:, j * 256:(j + 1) * 256], AF.Square,
                                     accum_out=sqsum[:, j:j + 1])
            nc.vector.tensor_reduce(
                ssum[:, 0:nblk], t[:].rearrange("p (n c) -> p n c", n=nblk),
                axis=mybir.AxisListType.X, op=OP.add)
            # var*256 = sqsum - ssum^2/256 ; std = sqrt(var + eps) via act scale
            nc.vector.scalar_tensor_tensor(nm2[:, 0:nblk], ssum[:, 0:nblk], 1.0 / 256.0,
                                           ssum[:, 0:nblk], op0=OP.mult, op1=OP.mult)
            nc.vector.tensor_tensor(var[:, 0:nblk], sqsum[:, 0:nblk], nm2[:, 0:nblk],
                                    op=OP.subtract)
            nc.scalar.activation(std[:, 0:nblk], var[:, 0:nblk], AF.Sqrt,
                                 scale=1.0 / 256.0, bias=epst[:])
            nc.vector.reciprocal(rstd[:, 0:nblk], std[:, 0:nblk])
            nc.vector.scalar_tensor_tensor(nmr[:, 0:nblk], ssum[:, 0:nblk], -1.0 / 256.0,
                                           rstd[:, 0:nblk], op0=OP.mult, op1=OP.mult)
            for j in range(nblk):
                emit_out(j, rstd[:, j:j + 1], nmr[:, j:j + 1])

        def xtcol(j, dh):
            # bh-major layout: (bh, dh, t) columns
            return (j // 4) * 1024 + dh * 512 + (j % 4) * 128

        def transpose_to(dstT, src):
            """dstT[(bh,dh,t)] = src^T: 4 transposes per PSUM bank, one wide
            copy per (bh,dh) group (destinations are contiguous 512 cols)."""
            for bh in range(2):
                for dh in range(2):
                    pt = ps.tile([128, 512], bf16, tag="hot1", bufs=2, name="pt")
                    for i in range(4):
                        j = bh * 4 + i
                        nc.tensor.transpose(
                            pt[:, i * 128:(i + 1) * 128],
                            src[:, j * 256 + dh * 128:j * 256 + dh * 128 + 128],
                            ident[:])
                    nc.vector.tensor_copy(
                        dstT[:, bh * 1024 + dh * 512:bh * 1024 + dh * 512 + 512],
                        pt[:])

        def make_xT():
            transpose_to(xTbf, xbf)

        # ---------------- embedding ----------------
        with tc.tile_pool(name="embp", bufs=1) as embp:
            ohsb = [embp.tile([128, 1024], f32, tag=f"oh{v}", name=f"oh{v}") for v in range(2)]
            embsb = [embp.tile([128, 256], f32, tag=f"em{v}", name=f"em{v}") for v in range(2)]
            for v in range(2):
                nc.sync.dma_start(ohsb[v][:], oh[v * 128:(v + 1) * 128, :])
                nc.sync.dma_start(embsb[v][:], emb[v * 128:(v + 1) * 128, :])
            for j in range(NBT):
                pe = ps.tile([128, 512], f32, tag="hot0", bufs=2, name="pe")
                for v in range(2):
                    nc.tensor.matmul(pe[:, 0:256], ohsb[v][:, j * 128:(j + 1) * 128],
                                     embsb[v][:], start=(v == 0), stop=(v == 1))
                nc.scalar.copy(zst[:, j * 256:(j + 1) * 256], pe[:, 0:256])

        def emit_xf(j, r, b):
            nc.scalar.activation(xf[:, j * 256:(j + 1) * 256],
                                 zst[:, j * 256:(j + 1) * 256], AF.Identity,
                                 scale=r, bias=b)
        ln_batch(zst, NBT, emit_xf)
        nc.vector.tensor_copy(xbf[:], xf[:])
        make_xT()
        if taps:
            nc.sync.dma_start(tap_t["t_x0"][:], xf[:])

        # ---------------- layers ----------------
        for lay in range(n_layer):
            # -- P1: xs^T = relu(wenc^T x) --
            for lt2 in range(NLT // 2):
                wt = wp.tile([128, 512], bf16, tag="wenc")
                nc.sync.dma_start(
                    wt[:].rearrange("p (q c) -> p q c", q=2),
                    wenc[lt2 * 256:(lt2 + 1) * 256, :].rearrange(
                        "(q p) c -> p q c", q=2))
                for sub in range(2):
                    lt = lt2 * 2 + sub
                    for bh in range(2):  # bt halves (512 cols each)
                        pp = ps.tile([128, 512], f32, tag="hot0", bufs=2, name="pp")
                        for dh in range(2):
                            nc.tensor.matmul(
                                pp[:], wt[:, sub * 256 + dh * 128:sub * 256 + dh * 128 + 128],
                                xTbf[:, bh * 1024 + dh * 512:bh * 1024 + dh * 512 + 512],
                                start=(dh == 0), stop=(dh == 1))
                        dst = xs[:, lt * 1024 + bh * 512:lt * 1024 + bh * 512 + 512]
                        if lt % 2 == 0:
                            nc.scalar.activation(dst, pp[:], AF.Relu)
                        else:
                            nc.vector.tensor_scalar_max(dst, pp[:], 0.0)
            if taps and lay == 0:
                nc.sync.dma_start(tap_t["t_xs"][:], xs[:, 0:1024])

            # -- P2+P3: rope -> scores -> attn halves (per b) --
            for b in ([] if ablate == "att" else range(B)):
                psc = [pacc.tile([128, 512], f32, tag=f"acc{u}", bufs=1, name=f"psc{u}") for u in range(4)]
                for pt in range(16):  # pair tiles
                    tg = tp.tile([128, 1024], bf16, tag="trig")
                    nc.sync.dma_start(tg[:], trig[pt * 128:(pt + 1) * 128, :])
                    ct, st = tg[:, 0:512], tg[:, 512:1024]
                    xe = xs[:, pt * 1024 + b * 512:pt * 1024 + b * 512 + 512]
                    xo = xs[:, (16 + pt) * 1024 + b * 512:(16 + pt) * 1024 + b * 512 + 512]
                    t1 = sm.tile([128, 512], bf16, tag="ropet1", bufs=2)
                    t2 = sm.tile([128, 512], bf16, tag="ropet2", bufs=2)
                    t3 = sm.tile([128, 512], bf16, tag="ropet3", bufs=2)
                    t4 = sm.tile([128, 512], bf16, tag="ropet4", bufs=2)
                    q2 = qp.tile([128, 1024], bf16, tag="q2")
                    nc.vector.tensor_tensor(t1[:], xe, ct, op=OP.mult)
                    nc.vector.tensor_tensor(t2[:], xo, st, op=OP.mult)
                    nc.vector.tensor_tensor(q2[:, 0:512], t1[:], t2[:],
                                            op=OP.subtract)
                    nc.gpsimd.tensor_tensor(t3[:], xo, ct, op=OP.mult)
                    nc.gpsimd.tensor_tensor(t4[:], xe, st, op=OP.mult)
                    nc.gpsimd.tensor_tensor(q2[:, 512:1024], t3[:], t4[:],
                                            op=OP.add)
                    for ki in range(2):
                        qt = q2[:, ki * 512:(ki + 1) * 512]
                        for ut in range(4):
                            n = 512 - ut * 128
                            nc.tensor.matmul(
                                psc[ut][:, 0:n], qt[:, ut * 128:(ut + 1) * 128],
                                qt[:, ut * 128:512],
                                start=(pt == 0 and ki == 0),
                                stop=(pt == 15 and ki == 1))
                # evict scores (mask diag), bf16
                off = 0
                for ut in range(4):
                    n = 512 - ut * 128
                    nc.vector.tensor_tensor(scsb[b][:, off:off + 128],
                                            psc[ut][:, 0:128], maskU[:], op=OP.mult)
                    if n > 128:
                        nc.scalar.copy(scsb[b][:, off + 128:off + n],
                                       psc[ut][:, 128:n])
                    off += n
                # attn half: [t-tile, d] accumulated over u tiles
                for tt in range(4):
                    pa = ps.tile([128, 512], f32, tag="hot1", bufs=2, name="pa")
                    for ut in range(tt + 1):
                        off = sum(512 - 128 * j for j in range(ut))
                        lhs = scsb[b][:, off + (tt - ut) * 128:off + (tt - ut) * 128 + 128]
                        rhs = xbf[:, (b * 4 + ut) * 256:(b * 4 + ut) * 256 + 256]
                        nc.tensor.matmul(pa[:, 0:256], lhs, rhs, start=(ut == 0),
                                         stop=(ut == tt))
                    j = b * 4 + tt
                    nc.scalar.copy(attn[:, j * 256:(j + 1) * 256], pa[:, 0:256])

            # -- pair AllReduce of attn halves (bf16) --
            if ablate == "att":
                nc.vector.memset(attn[:], 0.001)
            if ablate not in ("att", "noar"):
                a_src = dram.tile([1024, 256], bf16, tag="asrc")
                a_dst = dram.tile([1024, 256], bf16, tag="adst")
                nc.sync.dma_start(
                    a_src[:].rearrange("(j p) c -> p j c", j=NBT),
                    attn[:].rearrange("p (j c) -> p j c", j=NBT))
                nc.gpsimd.collective_compute(
                    "AllReduce", OP.add,
                    replica_groups=[[0, 1], [2, 3], [4, 5], [6, 7]],
                    ins=[a_src.opt()], outs=[a_dst.opt()])

            # -- P4: Hebbian chunks (overlaps the AR) --
            for c2 in range(B):
                nc.vector.memset(xsh[c2][96:128, :], 0.0)
                nc.sync.dma_start(xsh[c2][0:127, :],
                                  xbf[1:128, (c2 * 4) * 256:(c2 * 4) * 256 + 256])
            for k in ([] if ablate == "heb" else range(NCH)):
                # transposes: xbt[c2] partition p = xs time (k*128-1+p) [shifted]
                for c2 in range(B):
                    if k == 0:
                        nc.vector.memset(xbt[c2][96:128, :], 0.0)
                    for lt4 in range(NLT // 4):
                        pt = ps.tile([128, 512], bf16, tag="hot0", bufs=2, name="pt4")
                        for q4 in range(4):
                            lt = lt4 * 4 + q4
                            base = lt * 1024 + c2 * 512 + k * 128 - 1
                            if k == 0:
                                src = xs[:, lt * 1024 + c2 * 512:lt * 1024 + c2 * 512 + 127]
                                nc.tensor.transpose(pt[0:127, q4 * 128:q4 * 128 + 128],
                                                    src, ident[:])
                            else:
                                nc.tensor.transpose(pt[:, q4 * 128:q4 * 128 + 128],
                                                    xs[:, base:base + 128], ident[:])
                        rows = slice(0, 127) if k == 0 else slice(0, 128)
                        if lt4 % 2 == 0:
                            nc.vector.tensor_copy(
                                xbt[c2][rows, lt4 * 512:lt4 * 512 + 512], pt[rows, :])
                        else:
                            nc.scalar.copy(
                                xbt[c2][rows, lt4 * 512:lt4 * 512 + 512], pt[rows, :])
                mS = maskS0 if k == 0 else maskS
                # S^T blocks [u, (c2, b, t)] for both b in one bank (b-merged
                # rhs via 3D AP halves the matmul count)
                pst = ps.tile([128, 512], f32, tag="hot1", bufs=2, name="pst")
                for c2 in range(B):
                    for lt in range(NLT):
                        if k == 0:
                            lhs = xs[:, lt * 1024 + c2 * 512:lt * 1024 + c2 * 512 + 127]
                            m = 127
                        else:
                            base = lt * 1024 + c2 * 512 + k * 128 - 1
                            lhs = xs[:, base:base + 128]
                            m = 128
                        rhs3 = xs[:, lt * 1024:(lt + 1) * 1024].rearrange(
                            "p (b n) -> p b n", b=2)[:, :, k * 128:k * 128 + 128]
                        nc.tensor.matmul(
                            pst[0:m, c2 * 256:c2 * 256 + 256].rearrange(
                                "p (b t) -> p b t", b=2),
                            lhs, rhs3,
                            start=(lt == 0), stop=(lt == NLT - 1))
                for b in range(B):
                    for c2 in range(B):
                        nc.vector.tensor_tensor(
                            ssb[b][:, c2 * 128:c2 * 128 + 128],
                            pst[:, c2 * 256 + b * 128:c2 * 256 + b * 128 + 128],
                            mS[:], op=OP.mult)
                for b in range(B):
                    # H term + S apply into one psum bank
                    ph = pacc.tile([128, 512], f32, tag=f"acc{b}", bufs=1, name="ph")
                    for lt in range(NLT):
                        nc.tensor.matmul(
                            ph[:, 0:256],
                            xs[:, lt * 1024 + b * 512 + k * 128:lt * 1024 + b * 512 + k * 128 + 128],
                            F[:, lt * 256:(lt + 1) * 256],
                            start=(lt == 0), stop=False)
                    for c2 in range(B):
                        rhs = (xsh[c2][:] if k == 0
                               else xbf[:, (c2 * 4 + k) * 256:(c2 * 4 + k) * 256 + 256])
                        nc.tensor.matmul(ph[:, 0:256], ssb[b][:, c2 * 128:c2 * 128 + 128],
                                         rhs, start=False, stop=(c2 == B - 1))
                    j = b * 4 + k
                    nc.scalar.copy(ymlp[:, j * 256:(j + 1) * 256], ph[:, 0:256])
                # dF and F update
                for lt in range(NLT):
                    pdf = ps.tile([128, 512], f32, tag="hot0", bufs=2, name="pdf")
                    for c2 in range(B):
                        rhs = (xsh[c2][:] if k == 0
                               else xbf[:, (c2 * 4 + k) * 256:(c2 * 4 + k) * 256 + 256])
                        nc.tensor.matmul(pdf[:, 0:256],
                                         xbt[c2][:, lt * 128:(lt + 1) * 128], rhs,
                                         start=(c2 == 0), stop=(c2 == B - 1))
                    nc.vector.scalar_tensor_tensor(
                        F[:, lt * 256:(lt + 1) * 256], pdf[:, 0:256], LR,
                        F[:, lt * 256:(lt + 1) * 256], op0=OP.mult, op1=OP.add)

            # -- P5: attn AR result -> LN -> y_kv -> y_kv^T --
            if ablate not in ("att", "noar"):
                nc.sync.dma_start(
                    attn[:].rearrange("p (j c) -> p j c", j=NBT),
                    a_dst[:].rearrange("(j p) c -> p j c", j=NBT))

            def emit_ykv(j, r, bb):
                nc.scalar.activation(ykv[:, j * 256:(j + 1) * 256],
                                     attn[:, j * 256:(j + 1) * 256], AF.Identity,
                                     scale=r, bias=bb)
            ln_batch(attn, NBT, emit_ykv)
            transpose_to(ykvT, ykv)

            # -- P6: ys, xy, decoder accumulation --
            pdec = [pacc.tile([128, 512], f32, tag=f"acc{i}", bufs=1, name=f"pdec{i}") for i in range(4)]
            for lt2 in range(NLT // 2):
                wv = wp.tile([128, 512], bf16, tag="wencv")
                nc.sync.dma_start(
                    wv[:].rearrange("p (q c) -> p q c", q=2),
                    wencv[lt2 * 256:(lt2 + 1) * 256, :].rearrange(
                        "(q p) c -> p q c", q=2))
                wd = wp.tile([128, 512], bf16, tag="wdec")
                nc.sync.dma_start(
                    wd[:].rearrange("p (q c) -> p q c", q=2),
                    wdec[lt2 * 256:(lt2 + 1) * 256, :].rearrange(
                        "(q p) c -> p q c", q=2))
                for sub in range(2):
                    lt = lt2 * 2 + sub
                    ys = ysp.tile([128, 1024], bf16, tag="ys")
                    for bh in range(2):
                        pp = ps.tile([128, 512], f32, tag="hot0", bufs=2, name="pp2")
                        for dh in range(2):
                            nc.tensor.matmul(
                                pp[:], wv[:, sub * 256 + dh * 128:sub * 256 + dh * 128 + 128],
                                ykvT[:, bh * 1024 + dh * 512:bh * 1024 + dh * 512 + 512],
                                start=(dh == 0), stop=(dh == 1))
                        dst = ys[:, bh * 512:(bh + 1) * 512]
                        if lt % 2 == 0:
                            nc.scalar.activation(dst, pp[:], AF.Relu)
                        else:
                            nc.vector.tensor_scalar_max(dst, pp[:], 0.0)
                    nc.gpsimd.tensor_tensor(ys[:], ys[:],
                                            xs[:, lt * 1024:(lt + 1) * 1024],
                                            op=OP.mult)
                    for j in range(NBT):
                        nc.tensor.matmul(
                            pdec[j // 2][:, (j % 2) * 256:(j % 2) * 256 + 256],
                            ys[:, j * 128:(j + 1) * 128],
                            wd[:, sub * 256:sub * 256 + 256],
                            start=(lt == 0), stop=(lt == NLT - 1))

            # -- P7: finalize y_mlp partial, 8-core AR, residual+LN --
            for i in range(4):
                if ablate == "heb":
                    nc.vector.tensor_copy(ymlp[:, i * 512:(i + 1) * 512],
                                          pdec[i][:, 0:512])
                else:
                    nc.vector.tensor_tensor(ymlp[:, i * 512:(i + 1) * 512],
                                            pdec[i][:, 0:512],
                                            ymlp[:, i * 512:(i + 1) * 512], op=OP.add)
            if ablate != "noar":
                # 8-core AllReduce via recursive doubling: 3 pair-stage ARs
                # (pair collectives are far cheaper than one flat 8-core AR
                # on this runtime).
                ybufs = [dram.tile([1024, 256], bf16, tag=f"yb{i}", name=f"yb{i}")
                         for i in range(4)]
                stages = [
                    [[0, 1], [2, 3], [4, 5], [6, 7]],
                    [[0, 2], [1, 3], [4, 6], [5, 7]],
                    [[0, 4], [1, 5], [2, 6], [3, 7]],
                ]
                nc.sync.dma_start(
                    ybufs[0][:].rearrange("(j p) c -> p j c", j=NBT),
                    ymlp[:].rearrange("p (j c) -> p j c", j=NBT))
                for s in range(3):
                    nc.gpsimd.collective_compute(
                        "AllReduce", OP.add, replica_groups=stages[s],
                        ins=[ybufs[s].opt()], outs=[ybufs[s + 1].opt()])
                nc.sync.dma_start(
                    ymlp[:].rearrange("p (j c) -> p j c", j=NBT),
                    ybufs[3][:].rearrange("(j p) c -> p j c", j=NBT))
            if taps and lay == 0:
                nc.sync.dma_start(tap_t["t_ymlp"][:], ymlp[:])
                nc.sync.dma_start(tap_t["t_attn"][:], attn[:])

            def emit_z(j, r, bb):
                nc.scalar.activation(zst[:, j * 256:(j + 1) * 256],
                                     ymlp[:, j * 256:(j + 1) * 256], AF.Identity,
                                     scale=r, bias=bb)
            ln_batch(ymlp, NBT, emit_z)
            nc.vector.tensor_tensor(zst[:], zst[:], xf[:], op=OP.add)
            ln_batch(zst, NBT, emit_xf)
            nc.vector.tensor_copy(xbf[:], xf[:])
            make_xT()
            if taps and lay == 0:
                nc.sync.dma_start(tap_t["t_x1"][:], xf[:])

        if taps:
            nc.sync.dma_start(tap_t["t_f"][:], F[:])

        # ---------------- lm head ----------------
        lsb = sm.tile([128, 260], bf16, tag="lmh", bufs=1)
        for dh in range(2):
            nc.sync.dma_start(lsb[:, dh * 130:(dh + 1) * 130],
                              lmh[dh * 128:(dh + 1) * 128, :])
        for j in range(NBT):
            pl = ps.tile([128, 512], f32, tag="hot0", bufs=2, name="pl")
            for dh in range(2):
                nc.tensor.matmul(pl[:, 0:130],
                                 xTbf[:, xtcol(j, dh):xtcol(j, dh) + 128],
                                 lsb[:, dh * 130:(dh + 1) * 130],
                                 start=(dh == 0), stop=(dh == 1))
            lg = sm.tile([128, 130], f32, tag="lg")
            nc.scalar.copy(lg[:], pl[:, 0:130])
            nc.sync.dma_start(out[j * 128:(j + 1) * 128, :], lg[:])

    return nc


def _get_nc(n_layer=N_LAYER, taps=False, ablate=None):
    key = (n_layer, taps, ablate)
    if key not in _CACHE:
        nc = bacc.Bacc("TRN2", target_bir_lowering=False, debug=False,
                       num_devices=NCORE)
        _emit(nc, n_layer, taps, ablate)
        nc.compile()
        _CACHE[key] = nc
    return _CACHE[key]


# -------------------------------------------------------------- host side --
def _perm_local():
    p = np.empty(SH, np.int64)
    p[:2048] = 2 * np.arange(2048)
    p[2048:] = 2 * np.arange(2048) + 1
    return p


def host_prep(idx, embed_w, encoder, encoder_v, decoder, lm_head):
    idx = np.asarray(idx).astype(np.int64)
    embed_w = np.asarray(embed_w, np.float32)
    encoder = np.asarray(encoder, np.float32)
    encoder_v = np.asarray(encoder_v, np.float32)
    decoder = np.asarray(decoder, np.float32)
    lm_head = np.asarray(lm_head, np.float32)
    perm = _perm_local()

    onehotT = np.zeros((256, 1024), np.float32)
    flat = idx.reshape(-1)
    onehotT[flat, np.arange(1024)] = 1.0
    embedp = np.zeros((256, 256), np.float32)
    embedp[:VOCAB] = embed_w

    masks = np.zeros((128, 384), np.float32)
    i = np.arange(128)
    masks[:, 0:128] = (i[None, :] > i[:, None]).astype(np.float32)      # maskU[u,t]
    masks[:, 128:256] = LR * (i[:, None] < i[None, :]).astype(np.float32)  # maskS
    # maskS0: partition p = u-1 (u=p+1); cond u<t -> p+1<t ; row 127 -> 0
    m0 = LR * ((i[:, None] + 1) < i[None, :]).astype(np.float32)
    m0[127] = 0.0
    masks[:, 256:384] = m0

    lmh = lm_head.astype(BF)

    in_maps = []
    for c in range(NCORE):
        h, half = c // 2, c % 2
        g = half * SH + perm
        we = encoder[h][:, g]            # [256, 4096]
        wv = encoder_v[h][:, g]
        dec = decoder[h * L + g, :]      # [4096, 256]
        # tile wenc/wencv: [lt*128 + p, dh*128 + c] = we[dh*128+p, lt*128+c]
        wet = np.ascontiguousarray(
            we.reshape(2, 128, NLT, 128).transpose(2, 1, 0, 3)
            .reshape(NLT * 128, 256)).astype(BF)
        wvt = np.ascontiguousarray(
            wv.reshape(2, 128, NLT, 128).transpose(2, 1, 0, 3)
            .reshape(NLT * 128, 256)).astype(BF)
        pg = half * 2048 + np.arange(2048)
        freq = (1.0 / (THETA ** ((2.0 * pg) / L))) / TWO_PI
        ph = np.mod(np.arange(T)[None, :].astype(np.float64)
                    * freq[:, None].astype(np.float64), 1.0) * TWO_PI
        trig = np.concatenate([np.cos(ph), np.sin(ph)], 1).astype(BF)  # [2048,1024]
        in_maps.append({
            "onehotT": onehotT, "embedp": embedp,
            "wenc": wet, "wencv": wvt,
            "wdec": np.ascontiguousarray(dec).astype(BF),
            "trig": np.ascontiguousarray(trig),
            "masks": masks, "lmh": lmh,
        })
    return in_maps


def kernel(idx, embed_w, encoder, encoder_v, decoder, lm_head,
           n_layer=N_LAYER, taps=False, ablate=None, _return_raw=False):
    in_maps = host_prep(idx, embed_w, encoder, encoder_v, decoder, lm_head)
    nc = _get_nc(n_layer, taps, ablate)
    r = run_bass_kernel_spmd(nc, in_maps, core_ids=list(range(NCORE)))
    if _return_raw:
        return r
    return np.ascontiguousarray(
        r.results[0]["out"].reshape(B, T, VOCAB).astype(np.float32))


# revision 38
# speedup vs baseline: 1.0643x; 1.0643x over previous
"""Trainium2 Bass kernel for nn_BDH_90984587198975 (6-layer BDH with Hebbian
fast weights), SPMD over 8 NeuronCores.

Sharding: tensor-parallel over the flattened latent dim NHL=4*8192.  Core c
owns a 4096-wide slice of head h=c//2 (half=c%2), with lanes permuted so rope
pairs split into [even-members(2048) | odd-members(2048)] (rotation becomes a
tile swap instead of a cross-partition shuffle).  F (fast weights) stays
sharded by latent rows — its update is local.  Per layer there are exactly two
collectives: a pair AllReduce of the attention partial (L split in half within
a head) and an 8-core AllReduce of the y_mlp partial (decoder/F/Hebbian terms
contract over the latent shard).

All matmuls run in bf16 (f32 accumulation in PSUM); LayerNorm statistics and
the residual stream stay f32.
"""
import math
import numpy as np
import ml_dtypes

import concourse.bass as bass
import concourse.mybir as mybir
import concourse.tile as tile
from concourse import bacc
from concourse.masks import make_identity
from concourse.bass_utils import run_bass_kernel_spmd

BF = ml_dtypes.bfloat16
F8 = ml_dtypes.float8_e4m3
f32 = mybir.dt.float32
bf16 = mybir.dt.bfloat16
f8 = mybir.dt.float8e4
AF = mybir.ActivationFunctionType
OP = mybir.AluOpType
DR = mybir.MatmulPerfMode.DoubleRow
WSC = 32.0         # fp8 weight prescale (encoder/encoder_v/lm_head std=0.02)

N_LAYER = 6
D = 256
NH = 4
VOCAB = 130
LR = 0.01
L = 8192
EPS = 1e-5
TWO_PI = 2.0 * math.pi
THETA = 65536.0
B, T = 2, 512
NCORE = 8
SH = 4096          # latent shard per core
NLT = SH // 128    # 32 latent tiles
NBT = (B * T) // 128  # 8 bt tiles
CHK = 128          # hebbian time chunk
NCH = T // CHK     # 4 chunks

_CACHE = {}


# ----------------------------------------------------------------- builder --
def _emit(nc, n_layer, taps, ablate=None):
    # ---- DRAM I/O ----
    oh = nc.dram_tensor("onehotT", [256, 1024], f32, kind="ExternalInput")
    emb = nc.dram_tensor("embedp", [256, 256], f32, kind="ExternalInput")
    wenc = nc.dram_tensor("wenc", [NLT * 128, 256], bf16, kind="ExternalInput")
    wencv = nc.dram_tensor("wencv", [NLT * 128, 256], bf16, kind="ExternalInput")
    wdec = nc.dram_tensor("wdec", [SH, 256], bf16, kind="ExternalInput")
    trig = nc.dram_tensor("trig", [2048, 1024], bf16, kind="ExternalInput")
    msk = nc.dram_tensor("masks", [128, 384], f32, kind="ExternalInput")
    lmh = nc.dram_tensor("lmh", [256, 130], bf16, kind="ExternalInput")
    out = nc.dram_tensor("out", [1024, 130], f32, kind="ExternalOutput")
    tap_t = {}
    if taps:
        for name, shape, dt_ in [("t_x0", [128, 2048], f32), ("t_xs", [128, 1024], bf16),
                                 ("t_attn", [128, 2048], bf16), ("t_ymlp", [128, 2048], bf16),
                                 ("t_x1", [128, 2048], f32), ("t_f", [128, 8192], bf16)]:
            tap_t[name] = nc.dram_tensor(name, shape, dt_, kind="ExternalOutput")

    from contextlib import ExitStack
    tc = tile.TileContext(nc)
    with tc, ExitStack() as stk:
        per = stk.enter_context(tc.tile_pool(name="per", bufs=1))
        sm = stk.enter_context(tc.tile_pool(name="sm", bufs=2))
        qp = stk.enter_context(tc.tile_pool(name="qp", bufs=4))
        ysp = stk.enter_context(tc.tile_pool(name="ysp", bufs=3))
        wp = stk.enter_context(tc.tile_pool(name="wp", bufs=4))
        tp = stk.enter_context(tc.tile_pool(name="tp", bufs=3))
        ps = stk.enter_context(tc.tile_pool(name="ps", bufs=4, space="PSUM"))
        pacc = stk.enter_context(tc.tile_pool(name="pacc", bufs=4, space="PSUM"))
        dram = stk.enter_context(tc.tile_pool(name="dram", bufs=2, space="DRAM"))

        # ---- persistent tiles ----
        xs = per.tile([128, NLT * 1024], bf16, tag="xs")
        F = per.tile([128, NLT * 256], bf16, tag="F")
        xf = per.tile([128, 2048], f32, tag="xf")
        xbf = per.tile([128, 2048], bf16, tag="xbf")
        xTbf = per.tile([128, 2048], bf16, tag="xTbf")
        ymlp = per.tile([128, 2048], bf16, tag="ymlp")
        attn = per.tile([128, 2048], bf16, tag="attn")
        ykv = per.tile([128, 2048], bf16, tag="ykv")
        ykvT = per.tile([128, 2048], bf16, tag="ykvT")
        zst = per.tile([128, 2048], f32, tag="zst")
        ident = per.tile([128, 128], bf16, tag="ident")
        maskU = per.tile([128, 128], f32, tag="maskU")
        maskS = per.tile([128, 128], f32, tag="maskS")
        maskS0 = per.tile([128, 128], f32, tag="maskS0")
        epst = per.tile([128, 1], f32, tag="epst")
        xsh = [per.tile([128, 256], bf16, tag=f"xsh{c}", name=f"xsh{c}") for c in range(B)]
        xbt = [per.tile([128, SH], bf16, tag=f"xbt{c}", name=f"xbt{c}") for c in range(B)]
        scsb = [per.tile([128, 1280], bf16, tag=f"scsb{b}", name=f"scsb{b}") for b in range(B)]
        ssb = [per.tile([128, 256], bf16, tag=f"ssb{b}", name=f"ssb{b}") for b in range(B)]

        make_identity(nc, ident[:])
        nc.vector.memset(epst[:], EPS)
        nc.sync.dma_start(maskU[:], msk[:, 0:128])
        nc.sync.dma_start(maskS[:], msk[:, 128:256])
        nc.sync.dma_start(maskS0[:], msk[:, 256:384])
        nc.vector.memset(F[:], 0.0)

        # ---------------- helpers ----------------
        def ln_batch(t, nblk, emit_out):
            """Batched LN over `nblk` 256-wide blocks of tile t [128, nblk*256].
            emit_out(j, rstd_ap, nmr_ap) writes the normalized outputs."""
            ssum = sm.tile([128, 8], f32, tag="ssum")
            sqsum = sm.tile([128, 8], f32, tag="sqsum")
            nmean = sm.tile([128, 8], f32, tag="nmean")
            nm2 = sm.tile([128, 8], f32, tag="nm2")
            var = sm.tile([128, 8], f32, tag="var")
            std = sm.tile([128, 8], f32, tag="std")
            rstd = sm.tile([128, 8], f32, tag="rstd")
            nmr = sm.tile([128, 8], f32, tag="nmr")
            sq = sm.tile([128, 256], f32, tag="sq")
            for j in range(nblk):
                nc.scalar.activation(sq[:], t[:, j * 256:(j + 1) * 256], AF.Square,
                                     accum_out=sqsum[:, j:j + 1])
            nc.vector.tensor_reduce(
                ssum[:, 0:nblk], t[:].rearrange("p (n c) -> p n c", n=nblk),
                axis=mybir.AxisListType.X, op=OP.add)
            nc.vector.tensor_scalar_mul(nmean[:, 0:nblk], ssum[:, 0:nblk], -1.0 / 256.0)
            nc.vector.tensor_tensor(nm2[:, 0:nblk], nmean[:, 0:nblk], nmean[:, 0:nblk],
                                    op=OP.mult)
            nc.vector.scalar_tensor_tensor(var[:, 0:nblk], sqsum[:, 0:nblk], 1.0 / 256.0,
                                           nm2[:, 0:nblk], op0=OP.mult, op1=OP.subtract)
            nc.scalar.activation(std[:, 0:nblk], var[:, 0:nblk], AF.Sqrt, bias=epst[:])
            nc.vector.reciprocal(rstd[:, 0:nblk], std[:, 0:nblk])
            nc.vector.tensor_tensor(nmr[:, 0:nblk], nmean[:, 0:nblk], rstd[:, 0:nblk],
                                    op=OP.mult)
            for j in range(nblk):
                emit_out(j, rstd[:, j:j + 1], nmr[:, j:j + 1])

        def transpose_128(dst_ap, src_ap):
            pt = ps.tile([128, 512], bf16, tag="hot1", bufs=2, name="pt")
            nc.tensor.transpose(pt[:, 0:128], src_ap, ident[:])
            nc.vector.tensor_copy(dst_ap, pt[:, 0:128])

        def xtcol(j, dh):
            # bh-major layout so DoubleRow k-pairs (dh) sit 512 apart
            return (j // 4) * 1024 + dh * 512 + (j % 4) * 128

        def make_xT():
            """xTbf[(bh,dh,t)] = x^T from xbf (bf16 transposes, fp8 store)."""
            for j in range(NBT):
                for dh in range(2):
                    transpose_128(
                        xTbf[:, xtcol(j, dh):xtcol(j, dh) + 128],
                        xbf[:, j * 256 + dh * 128:j * 256 + dh * 128 + 128])

        # ---------------- embedding ----------------
        with tc.tile_pool(name="embp", bufs=1) as embp:
            ohsb = [embp.tile([128, 1024], f32, tag=f"oh{v}", name=f"oh{v}") for v in range(2)]
            embsb = [embp.tile([128, 256], f32, tag=f"em{v}", name=f"em{v}") for v in range(2)]
            for v in range(2):
                nc.sync.dma_start(ohsb[v][:], oh[v * 128:(v + 1) * 128, :])
                nc.sync.dma_start(embsb[v][:], emb[v * 128:(v + 1) * 128, :])
            for j in range(NBT):
                pe = ps.tile([128, 512], f32, tag="hot0", bufs=2, name="pe")
                for v in range(2):
                    nc.tensor.matmul(pe[:, 0:256], ohsb[v][:, j * 128:(j + 1) * 128],
                                     embsb[v][:], start=(v == 0), stop=(v == 1))
                nc.scalar.copy(zst[:, j * 256:(j + 1) * 256], pe[:, 0:256])

        def emit_xf(j, r, b):
            nc.scalar.activation(xf[:, j * 256:(j + 1) * 256],
                                 zst[:, j * 256:(j + 1) * 256], AF.Identity,
                                 scale=r, bias=b)
        ln_batch(zst, NBT, emit_xf)
        nc.vector.tensor_copy(xbf[:], xf[:])
        make_xT()
        if taps:
            nc.sync.dma_start(tap_t["t_x0"][:], xf[:])

        # ---------------- layers ----------------
        for lay in range(n_layer):
            # -- P1: xs^T = relu(wenc^T x) --
            for lt2 in range(NLT // 2):
                wt = wp.tile([128, 512], bf16, tag="wenc")
                nc.sync.dma_start(
                    wt[:].rearrange("p (q c) -> p q c", q=2),
                    wenc[lt2 * 256:(lt2 + 1) * 256, :].rearrange(
                        "(q p) c -> p q c", q=2))
                for sub in range(2):
                    lt = lt2 * 2 + sub
                    for bh in range(2):  # bt halves (512 cols each)
                        pp = ps.tile([128, 512], f32, tag="hot0", bufs=2, name="pp")
                        for dh in range(2):
                            nc.tensor.matmul(
                                pp[:], wt[:, sub * 256 + dh * 128:sub * 256 + dh * 128 + 128],
                                xTbf[:, bh * 1024 + dh * 512:bh * 1024 + dh * 512 + 512],
                                start=(dh == 0), stop=(dh == 1))
                        dst = xs[:, lt * 1024 + bh * 512:lt * 1024 + bh * 512 + 512]
                        if lt % 2 == 0:
                            nc.scalar.activation(dst, pp[:], AF.Relu)
                        else:
                            nc.vector.tensor_scalar_max(dst, pp[:], 0.0)
            if taps and lay == 0:
                nc.sync.dma_start(tap_t["t_xs"][:], xs[:, 0:1024])

            # -- P2+P3: rope -> scores -> attn halves (per b) --
            for b in ([] if ablate == "att" else range(B)):
                psc = [pacc.tile([128, 512], f32, tag=f"acc{u}", bufs=1, name=f"psc{u}") for u in range(4)]
                for pt in range(16):  # pair tiles
                    tg = tp.tile([128, 1024], bf16, tag="trig")
                    nc.sync.dma_start(tg[:], trig[pt * 128:(pt + 1) * 128, :])
                    ct, st = tg[:, 0:512], tg[:, 512:1024]
                    xe = xs[:, pt * 1024 + b * 512:pt * 1024 + b * 512 + 512]
                    xo = xs[:, (16 + pt) * 1024 + b * 512:(16 + pt) * 1024 + b * 512 + 512]
                    t1 = sm.tile([128, 512], bf16, tag="ropet1", bufs=2)
                    t2 = sm.tile([128, 512], bf16, tag="ropet2", bufs=2)
                    t3 = sm.tile([128, 512], bf16, tag="ropet3", bufs=2)
                    t4 = sm.tile([128, 512], bf16, tag="ropet4", bufs=2)
                    q2 = qp.tile([128, 1024], bf16, tag="q2")
                    nc.vector.tensor_tensor(t1[:], xe, ct, op=OP.mult)
                    nc.vector.tensor_tensor(t2[:], xo, st, op=OP.mult)
                    nc.vector.tensor_tensor(q2[:, 0:512], t1[:], t2[:],
                                            op=OP.subtract)
                    nc.gpsimd.tensor_tensor(t3[:], xo, ct, op=OP.mult)
                    nc.gpsimd.tensor_tensor(t4[:], xe, st, op=OP.mult)
                    nc.gpsimd.tensor_tensor(q2[:, 512:1024], t3[:], t4[:],
                                            op=OP.add)
                    for ki in range(2):
                        qt = q2[:, ki * 512:(ki + 1) * 512]
                        for ut in range(4):
                            n = 512 - ut * 128
                            nc.tensor.matmul(
                                psc[ut][:, 0:n], qt[:, ut * 128:(ut + 1) * 128],
                                qt[:, ut * 128:512],
                                start=(pt == 0 and ki == 0),
                                stop=(pt == 15 and ki == 1))
                # evict scores (mask diag), bf16
                off = 0
                for ut in range(4):
                    n = 512 - ut * 128
                    nc.vector.tensor_tensor(scsb[b][:, off:off + 128],
                                            psc[ut][:, 0:128], maskU[:], op=OP.mult)
                    if n > 128:
                        nc.scalar.copy(scsb[b][:, off + 128:off + n],
                                       psc[ut][:, 128:n])
                    off += n
                # attn half: [t-tile, d] accumulated over u tiles
                for tt in range(4):
                    pa = ps.tile([128, 512], f32, tag="hot1", bufs=2, name="pa")
                    for ut in range(tt + 1):
                        off = sum(512 - 128 * j for j in range(ut))
                        lhs = scsb[b][:, off + (tt - ut) * 128:off + (tt - ut) * 128 + 128]
                        rhs = xbf[:, (b * 4 + ut) * 256:(b * 4 + ut) * 256 + 256]
                        nc.tensor.matmul(pa[:, 0:256], lhs, rhs, start=(ut == 0),
                                         stop=(ut == tt))
                    j = b * 4 + tt
                    nc.scalar.copy(attn[:, j * 256:(j + 1) * 256], pa[:, 0:256])

            # -- pair AllReduce of attn halves (bf16) --
            if ablate == "att":
                nc.vector.memset(attn[:], 0.001)
            if ablate not in ("att", "noar"):
                a_src = dram.tile([1024, 256], bf16, tag="asrc")
                a_dst = dram.tile([1024, 256], bf16, tag="adst")
                nc.sync.dma_start(
                    a_src[:].rearrange("(j p) c -> p j c", j=NBT),
                    attn[:].rearrange("p (j c) -> p j c", j=NBT))
                nc.gpsimd.collective_compute(
                    "AllReduce", OP.add,
                    replica_groups=[[0, 1], [2, 3], [4, 5], [6, 7]],
                    ins=[a_src.opt()], outs=[a_dst.opt()])

            # -- P4: Hebbian chunks (overlaps the AR) --
            for c2 in range(B):
                nc.vector.memset(xsh[c2][96:128, :], 0.0)
                nc.sync.dma_start(xsh[c2][0:127, :],
                                  xbf[1:128, (c2 * 4) * 256:(c2 * 4) * 256 + 256])
            for k in ([] if ablate == "heb" else range(NCH)):
                # transposes: xbt[c2] partition p = xs time (k*128-1+p) [shifted]
                for c2 in range(B):
                    if k == 0:
                        nc.vector.memset(xbt[c2][96:128, :], 0.0)
                    for lt4 in range(NLT // 4):
                        pt = ps.tile([128, 512], bf16, tag="hot0", bufs=2, name="pt4")
                        for q4 in range(4):
                            lt = lt4 * 4 + q4
                            base = lt * 1024 + c2 * 512 + k * 128 - 1
                            if k == 0:
                                src = xs[:, lt * 1024 + c2 * 512:lt * 1024 + c2 * 512 + 127]
                                nc.tensor.transpose(pt[0:127, q4 * 128:q4 * 128 + 128],
                                                    src, ident[:])
                            else:
                                nc.tensor.transpose(pt[:, q4 * 128:q4 * 128 + 128],
                                                    xs[:, base:base + 128], ident[:])
                        rows = slice(0, 127) if k == 0 else slice(0, 128)
                        if lt4 % 2 == 0:
                            nc.vector.tensor_copy(
                                xbt[c2][rows, lt4 * 512:lt4 * 512 + 512], pt[rows, :])
                        else:
                            nc.scalar.copy(
                                xbt[c2][rows, lt4 * 512:lt4 * 512 + 512], pt[rows, :])
                mS = maskS0 if k == 0 else maskS
                # S^T blocks [u, (c2, b, t)] for both b in one bank (b-merged
                # rhs via 3D AP halves the matmul count)
                pst = ps.tile([128, 512], f32, tag="hot1", bufs=2, name="pst")
                for c2 in range(B):
                    for lt in range(NLT):
                        if k == 0:
                            lhs = xs[:, lt * 1024 + c2 * 512:lt * 1024 + c2 * 512 + 127]
                            m = 127
                        else:
                            base = lt * 1024 + c2 * 512 + k * 128 - 1
                            lhs = xs[:, base:base + 128]
                            m = 128
                        rhs3 = xs[:, lt * 1024:(lt + 1) * 1024].rearrange(
                            "p (b n) -> p b n", b=2)[:, :, k * 128:k * 128 + 128]
                        nc.tensor.matmul(
                            pst[0:m, c2 * 256:c2 * 256 + 256].rearrange(
                                "p (b t) -> p b t", b=2),
                            lhs, rhs3,
                            start=(lt == 0), stop=(lt == NLT - 1))
                for b in range(B):
                    for c2 in range(B):
                        nc.vector.tensor_tensor(
                            ssb[b][:, c2 * 128:c2 * 128 + 128],
                            pst[:, c2 * 256 + b * 128:c2 * 256 + b * 128 + 128],
                            mS[:], op=OP.mult)
                for b in range(B):
                    # H term + S apply into one psum bank
                    ph = pacc.tile([128, 512], f32, tag=f"acc{b}", bufs=1, name="ph")
                    for lt in range(NLT):
                        nc.tensor.matmul(
                            ph[:, 0:256],
                            xs[:, lt * 1024 + b * 512 + k * 128:lt * 1024 + b * 512 + k * 128 + 128],
                            F[:, lt * 256:(lt + 1) * 256],
                            start=(lt == 0), stop=False)
                    for c2 in range(B):
                        rhs = (xsh[c2][:] if k == 0
                               else xbf[:, (c2 * 4 + k) * 256:(c2 * 4 + k) * 256 + 256])
                        nc.tensor.matmul(ph[:, 0:256], ssb[b][:, c2 * 128:c2 * 128 + 128],
                                         rhs, start=False, stop=(c2 == B - 1))
                    j = b * 4 + k
                    nc.scalar.copy(ymlp[:, j * 256:(j + 1) * 256], ph[:, 0:256])
                # dF and F update
                for lt in range(NLT):
                    pdf = ps.tile([128, 512], f32, tag="hot0", bufs=2, name="pdf")
                    for c2 in range(B):
                        rhs = (xsh[c2][:] if k == 0
                               else xbf[:, (c2 * 4 + k) * 256:(c2 * 4 + k) * 256 + 256])
                        nc.tensor.matmul(pdf[:, 0:256],
                                         xbt[c2][:, lt * 128:(lt + 1) * 128], rhs,
                                         start=(c2 == 0), stop=(c2 == B - 1))
                    nc.vector.scalar_tensor_tensor(
                        F[:, lt * 256:(lt + 1) * 256], pdf[:, 0:256], LR,
                        F[:, lt * 256:(lt + 1) * 256], op0=OP.mult, op1=OP.add)

            # -- P5: attn AR result -> LN -> y_kv -> y_kv^T --
            if ablate not in ("att", "noar"):
                nc.sync.dma_start(
                    attn[:].rearrange("p (j c) -> p j c", j=NBT),
                    a_dst[:].rearrange("(j p) c -> p j c", j=NBT))

            def emit_ykv(j, r, bb):
                nc.scalar.activation(ykv[:, j * 256:(j + 1) * 256],
                                     attn[:, j * 256:(j + 1) * 256], AF.Identity,
                                     scale=r, bias=bb)
            ln_batch(attn, NBT, emit_ykv)
            for j in range(NBT):
                for dh in range(2):
                    transpose_128(ykvT[:, xtcol(j, dh):xtcol(j, dh) + 128],
                                  ykv[:, j * 256 + dh * 128:j * 256 + dh * 128 + 128])

            # -- P6: ys, xy, decoder accumulation --
            pdec = [pacc.tile([128, 512], f32, tag=f"acc{i}", bufs=1, name=f"pdec{i}") for i in range(4)]
            for lt2 in range(NLT // 2):
                wv = wp.tile([128, 512], bf16, tag="wencv")
                nc.sync.dma_start(
                    wv[:].rearrange("p (q c) -> p q c", q=2),
                    wencv[lt2 * 256:(lt2 + 1) * 256, :].rearrange(
                        "(q p) c -> p q c", q=2))
                wd = wp.tile([128, 512], bf16, tag="wdec")
                nc.sync.dma_start(
                    wd[:].rearrange("p (q c) -> p q c", q=2),
                    wdec[lt2 * 256:(lt2 + 1) * 256, :].rearrange(
                        "(q p) c -> p q c", q=2))
                for sub in range(2):
                    lt = lt2 * 2 + sub
                    ys = ysp.tile([128, 1024], bf16, tag="ys")
                    for bh in range(2):
                        pp = ps.tile([128, 512], f32, tag="hot0", bufs=2, name="pp2")
                        for dh in range(2):
                            nc.tensor.matmul(
                                pp[:], wv[:, sub * 256 + dh * 128:sub * 256 + dh * 128 + 128],
                                ykvT[:, bh * 1024 + dh * 512:bh * 1024 + dh * 512 + 512],
                                start=(dh == 0), stop=(dh == 1))
                        dst = ys[:, bh * 512:(bh + 1) * 512]
                        if lt % 2 == 0:
                            nc.scalar.activation(dst, pp[:], AF.Relu)
                        else:
                            nc.vector.tensor_scalar_max(dst, pp[:], 0.0)
                    nc.gpsimd.tensor_tensor(ys[:], ys[:],
                                            xs[:, lt * 1024:(lt + 1) * 1024],
                                            op=OP.mult)
                    for j in range(NBT):
                        nc.tensor.matmul(
                            pdec[j // 2][:, (j % 2) * 256:(j % 2) * 256 + 256],
                            ys[:, j * 128:(j + 1) * 128],
                            wd[:, sub * 256:sub * 256 + 256],
                            start=(lt == 0), stop=(lt == NLT - 1))

            # -- P7: finalize y_mlp partial, 8-core AR, residual+LN --
            for i in range(4):
                if ablate == "heb":
                    nc.vector.tensor_copy(ymlp[:, i * 512:(i + 1) * 512],
                                          pdec[i][:, 0:512])
                else:
                    nc.vector.tensor_tensor(ymlp[:, i * 512:(i + 1) * 512],
                                            pdec[i][:, 0:512],
                                            ymlp[:, i * 512:(i + 1) * 512], op=OP.add)
            if ablate != "noar":
                # 8-core AllReduce via recursive doubling: 3 pair-stage ARs
                # (pair collectives are far cheaper than one flat 8-core AR
                # on this runtime).
                ybufs = [dram.tile([1024, 256], bf16, tag=f"yb{i}", name=f"yb{i}")
                         for i in range(4)]
                stages = [
                    [[0, 1], [2, 3], [4, 5], [6, 7]],
                    [[0, 2], [1, 3], [4, 6], [5, 7]],
                    [[0, 4], [1, 5], [2, 6], [3, 7]],
                ]
                nc.sync.dma_start(
                    ybufs[0][:].rearrange("(j p) c -> p j c", j=NBT),
                    ymlp[:].rearrange("p (j c) -> p j c", j=NBT))
                for s in range(3):
                    nc.gpsimd.collective_compute(
                        "AllReduce", OP.add, replica_groups=stages[s],
                        ins=[ybufs[s].opt()], outs=[ybufs[s + 1].opt()])
                nc.sync.dma_start(
                    ymlp[:].rearrange("p (j c) -> p j c", j=NBT),
                    ybufs[3][:].rearrange("(j p) c -> p j c", j=NBT))
            if taps and lay == 0:
                nc.sync.dma_start(tap_t["t_ymlp"][:], ymlp[:])
                nc.sync.dma_start(tap_t["t_attn"][:], attn[:])

            def emit_z(j, r, bb):
                nc.scalar.activation(zst[:, j * 256:(j + 1) * 256],
                                     ymlp[:, j * 256:(j + 1) * 256], AF.Identity,
                                     scale=r, bias=bb)
            ln_batch(ymlp, NBT, emit_z)
            nc.vector.tensor_tensor(zst[:], zst[:], xf[:], op=OP.add)
            ln_batch(zst, NBT, emit_xf)
            nc.vector.tensor_copy(xbf[:], xf[:])
            make_xT()
            if taps and lay == 0:
                nc.sync.dma_start(tap_t["t_x1"][:], xf[:])

        if taps:
            nc.sync.dma_start(tap_t["t_f"][:], F[:])

        # ---------------- lm head ----------------
        lsb = sm.tile([128, 260], bf16, tag="lmh", bufs=1)
        for dh in range(2):
            nc.sync.dma_start(lsb[:, dh * 130:(dh + 1) * 130],
                              lmh[dh * 128:(dh + 1) * 128, :])
        for j in range(NBT):
            pl = ps.tile([128, 512], f32, tag="hot0", bufs=2, name="pl")
            for dh in range(2):
                nc.tensor.matmul(pl[:, 0:130],
                                 xTbf[:, xtcol(j, dh):xtcol(j, dh) + 128],
                                 lsb[:, dh * 130:(dh + 1) * 130],
                                 start=(dh == 0), stop=(dh == 1))
            lg = sm.tile([128, 130], f32, tag="lg")
            nc.scalar.copy(lg[:], pl[:, 0:130])
            nc.sync.dma_start(out[j * 128:(j + 1) * 128, :], lg[:])

    return nc


def _get_nc(n_layer=N_LAYER, taps=False, ablate=None):
    key = (n_layer, taps, ablate)
    if key not in _CACHE:
        nc = bacc.Bacc("TRN2", target_bir_lowering=False, debug=False,
                       num_devices=NCORE)
        _emit(nc, n_layer, taps, ablate)
        nc.compile()
        _CACHE[key] = nc
    return _CACHE[key]


# -------------------------------------------------------------- host side --
def _perm_local():
    p = np.empty(SH, np.int64)
    p[:2048] = 2 * np.arange(2048)
    p[2048:] = 2 * np.arange(2048) + 1
    return p


def host_prep(idx, embed_w, encoder, encoder_v, decoder, lm_head):
    idx = np.asarray(idx).astype(np.int64)
    embed_w = np.asarray(embed_w, np.float32)
    encoder = np.asarray(encoder, np.float32)
    encoder_v = np.asarray(encoder_v, np.float32)
    decoder = np.asarray(decoder, np.float32)
    lm_head = np.asarray(lm_head, np.float32)
    perm = _perm_local()

    onehotT = np.zeros((256, 1024), np.float32)
    flat = idx.reshape(-1)
    onehotT[flat, np.arange(1024)] = 1.0
    embedp = np.zeros((256, 256), np.float32)
    embedp[:VOCAB] = embed_w

    masks = np.zeros((128, 384), np.float32)
    i = np.arange(128)
    masks[:, 0:128] = (i[None, :] > i[:, None]).astype(np.float32)      # maskU[u,t]
    masks[:, 128:256] = LR * (i[:, None] < i[None, :]).astype(np.float32)  # maskS
    # maskS0: partition p = u-1 (u=p+1); cond u<t -> p+1<t ; row 127 -> 0
    m0 = LR * ((i[:, None] + 1) < i[None, :]).astype(np.float32)
    m0[127] = 0.0
    masks[:, 256:384] = m0

    lmh = lm_head.astype(BF)

    in_maps = []
    for c in range(NCORE):
        h, half = c // 2, c % 2
        g = half * SH + perm
        we = encoder[h][:, g]            # [256, 4096]
        wv = encoder_v[h][:, g]
        dec = decoder[h * L + g, :]      # [4096, 256]
        # tile wenc/wencv: [lt*128 + p, dh*128 + c] = we[dh*128+p, lt*128+c]
        wet = np.ascontiguousarray(
            we.reshape(2, 128, NLT, 128).transpose(2, 1, 0, 3)
            .reshape(NLT * 128, 256)).astype(BF)
        wvt = np.ascontiguousarray(
            wv.reshape(2, 128, NLT, 128).transpose(2, 1, 0, 3)
            .reshape(NLT * 128, 256)).astype(BF)
        pg = half * 2048 + np.arange(2048)
        freq = (1.0 / (THETA ** ((2.0 * pg) / L))) / TWO_PI
        ph = np.mod(np.arange(T)[None, :].astype(np.float64)
                    * freq[:, None].astype(np.float64), 1.0) * TWO_PI
        trig = np.concatenate([np.cos(ph), np.sin(ph)], 1).astype(BF)  # [2048,1024]
        in_maps.append({
            "onehotT": onehotT, "embedp": embedp,
            "wenc": wet, "wencv": wvt,
            "wdec": np.ascontiguousarray(dec).astype(BF),
            "trig": np.ascontiguousarray(trig),
            "masks": masks, "lmh": lmh,
        })
    return in_maps


def kernel(idx, embed_w, encoder, encoder_v, decoder, lm_head,
           n_layer=N_LAYER, taps=False, ablate=None, _return_raw=False):
    in_maps = host_prep(idx, embed_w, encoder, encoder_v, decoder, lm_head)
    nc = _get_nc(n_layer, taps, ablate)
    r = run_bass_kernel_spmd(nc, in_maps, core_ids=list(range(NCORE)))
    if _return_raw:
        return r
    return np.ascontiguousarray(
        r.results[0]["out"].reshape(B, T, VOCAB).astype(np.float32))
